# revision 8
# baseline (speedup 1.0000x reference)
"""Trainium2 Bass kernel for ConcentrationLoss (fp8 streaming version).

Math (per batch element b, fully independent across b):
    g      = grid[b] viewed as (2, 4096)            # channels x pixels
    coord1 = g @ aff[b]                             # (2, 4096), the heavy op
    view coord1 as (2, 64, 64); extract 8x8 windows stride 4 -> 15x15 windows
    loss contribution = sum over windows w of [ sum_{p in w} x_p^2 - (sum_{p in w} x_p)^2 / 64 ]
    final = sum_b contribution_b / (8 * 2 * 225 * 64)

Sharding: batch b -> core b (8 cores).

The kernel is HBM-bandwidth bound: every byte of aff must be read once.
The loss tolerance (2e-2) is ~3500x above the baseline fp32 error, so aff
and g are quantized host-side to fp8 e4m3 (rel bias ~6e-3 on the loss,
measured against the fp32 oracle on CPU), cutting the stream from 64MB to
16MB per core. The matmul runs in DoubleRow fp8 mode (two contraction
rows per PE cell per cycle), so the PE (~31us) stays hidden under the
~45us DMA stream.

Device pipeline per core:
  - aff is host-packed into DMA-tile order [group, p, pair, i, n]: one
    2MB transfer per 512-pixel column group, fully contiguous, covering
    the full 4096-row contraction for that group.
  - Main matmul: for each group, 16 DoubleRow matmuls accumulate
    lhsT[128,2,2] (g chunk pairs) x rhs[128,2,512] (aff pair tiles) into
    one PSUM bank tile (2, 512).
  - When a bank finishes accumulating, its windowed reduction overlaps
    the remaining stream: square (ACT), then one 4-dim overlapping-AP
    tensor_reduce each for the w-direction window sums of x and x^2
    (DVE), then the newly-computable h-direction window rows.
  - The final (2, 2) output holds sum(SSq) and sum(S^2) per channel.
    Host: loss_b = sum_c [ sumSSq_c - sumS2_c / 64 ], all-reduced over
    the 8 cores on the host.
"""

import numpy as np

B = 8
C = 2
H = W = 64
PIX = H * W  # 4096, contraction dim
WIN = 8
STRIDE = 4
OH = OW = 15
KC = PIX // 128   # 32 contraction chunks of 128
NPAIR = KC // 2   # 16 DoubleRow chunk pairs
NT = PIX // 512   # 8 column groups == psum banks == DMA tiles
ROWS_PER_BANK = 512 // W  # 8 image rows per psum bank
GPAD = 16         # gt inner stride (pad 2 channels to 16B for DoubleRow LDW)
AFF_BUFS = 8      # 1MB half-tiles in flight

_CACHE = {}


def _split_multi_waits(nc, limit=1):
    """The walrus build in this toolchain rejects instructions carrying more
    than one sync wait (any template: CTRL, S3_LW, ...). Tile's scheduler
    freely emits multi-wait instructions. Post-process the scheduled BIR:
    hoist excess waits onto one-wait NoOps inserted immediately before the
    instruction on the same engine (sequencer waits are conjunctive and
    blocking, so semantics are identical)."""
    import concourse.mybir as mybir

    n_split = 0
    for f in nc.m.functions:
        for b in f.blocks:
            insts = b.instructions  # live view
            i = 0
            while i < len(insts):
                inst = insts[i]
                si = inst.sync_info
                if si is not None and len(si.on_wait) > limit:
                    waits = list(si.on_wait)
                    extra, keep = waits[:-limit], waits[-limit:]
                    for w in extra:
                        nop = mybir.InstNoOp(name=f"SWS-{n_split}")
                        n_split += 1
                        nop.engine = inst.engine
                        nop.sync_info = mybir.SyncInfo(on_wait=[w], on_update=[])
                        insts.insert(i, nop)
                        i += 1
                    inst.sync_info = mybir.SyncInfo(
                        on_wait=keep, on_update=si.on_update
                    )
                i += 1
    return n_split


def _build_nc(split=True):
    import concourse.bass as bass
    import concourse.mybir as mybir
    import concourse.tile as tile

    f32 = mybir.dt.float32
    f8 = mybir.dt.float8e4
    nc = bass.Bass()
    # aff pre-packed on the host: [group, half, p, pair, i, n]; element
    # (row, col) of aff[b] with row = 256*(8h+u) + 128*i + p, col = 512*g + n
    # lands at [g, h, p, u, i, n]. One contiguous 1MB transfer per half-group
    # so matmul bursts start on half-tile completion. The first two groups
    # stream through SWDGE (gpsimd), which starts draining ~6us before the
    # HWDGE rings come up; the rest through HWDGE (sync).
    aff = nc.dram_tensor(
        "aff", [NT, 2, 128, NPAIR // 2, 2, 512], f8, kind="ExternalInput"
    )
    # gt[p, kc, 0:2] = g[c, 128*kc + p]; inner dim padded to GPAD so the
    # DoubleRow weight pair stride is 16B
    gt = nc.dram_tensor("gt", [128, KC, GPAD], f8, kind="ExternalInput")
    out = nc.dram_tensor("out", [C, 2], f32, kind="ExternalOutput")

    with tile.TileContext(nc) as tc:
        with (
            tc.tile_pool(name="consts", bufs=1) as consts,
            tc.tile_pool(name="small", bufs=1) as small,
            tc.tile_pool(name="sqp", bufs=2) as sqp,
            tc.tile_pool(name="affp", bufs=AFF_BUFS) as affp,
            tc.tile_pool(name="ps1", bufs=1, space="PSUM") as ps1,
        ):
            # consts go through SWDGE (gpsimd) so they never queue behind the
            # big aff stream on the HWDGE rings
            gt_sb = consts.tile([128, KC, GPAD], f8)
            nc.gpsimd.dma_start(out=gt_sb, in_=gt[:, :, :])

            y_sb = small.tile([C, H, OW], f32)      # w-windowsums of x
            ysq_sb = small.tile([C, H, OW], f32)    # w-windowsums of x^2
            s_sb = small.tile([C, OH * OW], f32)    # full window sums
            ssq_sb = small.tile([C, OH * OW], f32)  # full window sums of x^2
            s2_sb = small.tile([C, OH * OW], f32)   # S^2
            out_sb = small.tile([C, 2], f32)

            def windowed(ap, row_step, n_rows):
                """4-dim overlapping AP: [part, row, window j, dw] over a
                (C, n_rows*row_step) region; one tensor_reduce(X) gives the
                w-direction window sums in a single instruction."""
                return bass.AP(
                    tensor=ap.tensor,
                    offset=ap.offset,
                    ap=[list(ap.ap[0]), [row_step, n_rows], [STRIDE, OW], [1, WIN]],
                )

            def bank_postprocess(n, bank):
                """w-direction window sums for psum bank n; overlaps stream."""
                sq = sqp.tile([C, 512], f32, tag="sq")
                nc.scalar.square(out=sq, in_=bank)
                yd = y_sb[:, n * ROWS_PER_BANK:(n + 1) * ROWS_PER_BANK, :]
                qd = ysq_sb[:, n * ROWS_PER_BANK:(n + 1) * ROWS_PER_BANK, :]
                nc.vector.reduce_sum(
                    out=yd, in_=windowed(bank, W, ROWS_PER_BANK),
                    axis=mybir.AxisListType.X,
                )
                nc.vector.reduce_sum(
                    out=qd, in_=windowed(sq[:, :], W, ROWS_PER_BANK),
                    axis=mybir.AxisListType.X,
                )

            # h-direction window sums, incremental: S[c, i, j] = sum_dh
            # Y[c, 4i+dh, j]. Window row i needs Y rows 4i..4i+7; after bank
            # n the rows up to 8n+7 exist, so rows {2n-1, 2n} (and row 0 for
            # n=0) become computable.
            sv = s_sb.rearrange("c (i j) -> c i j", j=OW)
            qv = ssq_sb.rearrange("c (i j) -> c i j", j=OW)

            def h_rows(i0, cnt):
                for src, dst in ((y_sb, sv), (ysq_sb, qv)):
                    ap = src[:, :, :]
                    win = bass.AP(
                        tensor=ap.tensor,
                        offset=ap.offset + i0 * STRIDE * OW,
                        ap=[list(ap.ap[0]), [STRIDE * OW, cnt], [1, OW], [OW, WIN]],
                    )
                    nc.vector.reduce_sum(
                        out=dst[:, i0:i0 + cnt, :], in_=win,
                        axis=mybir.AxisListType.X,
                    )

            # one PSUM bank tile per column group; group g covers pixels
            # [512g, 512g+512) == image rows [8g, 8g+8)
            c1bs = [
                ps1.tile([C, 512], f32, tag="bank", bufs=NT, name=f"c1b{g}")
                for g in range(NT)
            ]
            HP = NPAIR // 2  # pairs per half-tile
            for g in range(NT):
                for h in range(2):
                    at = affp.tile([128, HP, 2, 512], f8)
                    eng = nc.gpsimd if g < 2 else nc.sync
                    eng.dma_start(out=at, in_=aff[g, h])
                    for v in range(HP):
                        u = h * HP + v
                        nc.tensor.matmul(
                            c1bs[g],
                            lhsT=gt_sb[:, 2 * u:2 * u + 2, 0:2],
                            rhs=at[:, v, :, :],
                            start=(u == 0),
                            stop=(u == NPAIR - 1),
                            perf_mode=mybir.MatmulPerfMode.DoubleRow,
                        )
                bank_postprocess(g, c1bs[g])
                if g == 0:
                    h_rows(0, 1)
                else:
                    h_rows(2 * g - 1, 2)

            nc.scalar.square(out=s2_sb, in_=s_sb)
            nc.vector.reduce_sum(out=out_sb[:, 0:1], in_=ssq_sb, axis=mybir.AxisListType.X)
            nc.vector.reduce_sum(out=out_sb[:, 1:2], in_=s2_sb, axis=mybir.AxisListType.X)
            nc.sync.dma_start(out=out[:, :], in_=out_sb)
    if split:
        _split_multi_waits(nc)
    return nc


def _f8(x):
    import ml_dtypes

    return np.asarray(x, dtype=np.float32).astype(ml_dtypes.float8_e4m3)


def _gt_host(grid_b):
    # grid_b: (64, 64, 2). g[c, p] = grid_b.reshape(4096, 2)[p, c]
    # gt layout: gt[p, kc, c] = g[c, 128*kc + p], inner padded to GPAD
    import ml_dtypes

    g = np.ascontiguousarray(grid_b, dtype=np.float32).reshape(PIX, C)
    gt = np.zeros((128, KC, GPAD), dtype=ml_dtypes.float8_e4m3)
    gt[:, :, :C] = _f8(g.reshape(KC, 128, C).transpose(1, 0, 2))
    return gt


def _aff_host(aff_b):
    # pack into DMA-tile order [g, h, p, u, i, n]:
    # element (row, col) with row = 256*(8h+u) + 128i + p, col = 512g + n
    a8 = _f8(aff_b)  # (4096, 4096)
    a8 = a8.reshape(2, NPAIR // 2, 2, 128, NT, 512).transpose(4, 0, 3, 1, 2, 5)
    return np.ascontiguousarray(a8)


def run_cores(aff, grid, trace=False):
    """Compile (cached) and run the per-core bass kernel on cores 0..7.

    Returns the BassKernelResults from run_bass_kernel_spmd."""
    from concourse.bass_utils import run_bass_kernel_spmd

    if "nc" not in _CACHE:
        _CACHE["nc"] = _build_nc()
    nc = _CACHE["nc"]

    in_maps = []
    for b in range(B):
        in_maps.append({"aff": _aff_host(aff[b]), "gt": _gt_host(grid[b])})
    return run_bass_kernel_spmd(nc, in_maps, core_ids=list(range(B)), trace=trace)


def kernel(aff, grid):
    aff = np.asarray(aff, dtype=np.float32)
    grid = np.asarray(grid, dtype=np.float32)
    res = run_cores(aff, grid)
    total = 0.0
    for b in range(B):
        o = res.results[b]["out"].astype(np.float64)
        total += o[:, 0].sum() - o[:, 1].sum() / (WIN * WIN)
    total /= B * C * OH * OW * WIN * WIN
    return np.asarray(total, dtype=np.float32)


# revision 15
# speedup vs baseline: 1.3238x; 1.3238x over previous
"""Trainium2 Bass kernel for ConcentrationLoss (fp8 streaming version).

Math (per batch element b, fully independent across b):
    g      = grid[b] viewed as (2, 4096)            # channels x pixels
    coord1 = g @ aff[b]                             # (2, 4096), the heavy op
    view coord1 as (2, 64, 64); extract 8x8 windows stride 4 -> 15x15 windows
    loss contribution = sum over windows w of [ sum_{p in w} x_p^2 - (sum_{p in w} x_p)^2 / 64 ]
    final = sum_b contribution_b / (8 * 2 * 225 * 64)

Sharding: batch b -> core b (8 cores).

The kernel is HBM-bandwidth bound: every byte of aff must be read once.
The loss tolerance (2e-2) is ~3500x above the baseline fp32 error, so aff
and g are quantized host-side to fp8 e4m3 (rel bias ~6e-3 on the loss,
measured against the fp32 oracle on CPU), cutting the stream from 64MB to
16MB per core. The matmul runs in DoubleRow fp8 mode (two contraction
rows per PE cell per cycle), so the PE (~31us) stays hidden under the
~45us DMA stream.

Device pipeline per core:
  - aff is host-packed into DMA-tile order [group, p, pair, i, n]: one
    2MB transfer per 512-pixel column group, fully contiguous, covering
    the full 4096-row contraction for that group.
  - Main matmul: for each group, 16 DoubleRow matmuls accumulate
    lhsT[128,2,2] (g chunk pairs) x rhs[128,2,512] (aff pair tiles) into
    one PSUM bank tile (2, 512).
  - When a bank finishes accumulating, its windowed reduction overlaps
    the remaining stream: square (ACT), then one 4-dim overlapping-AP
    tensor_reduce each for the w-direction window sums of x and x^2
    (DVE), then the newly-computable h-direction window rows.
  - The final (2, 2) output holds sum(SSq) and sum(S^2) per channel.
    Host: loss_b = sum_c [ sumSSq_c - sumS2_c / 64 ], all-reduced over
    the 8 cores on the host.
"""

import numpy as np

B = 8
C = 2
H = W = 64
PIX = H * W  # 4096, contraction dim
WIN = 8
STRIDE = 4
OH = OW = 15
KC = PIX // 128   # 32 contraction chunks of 128
NPAIR = KC // 2   # 16 DoubleRow chunk pairs
NT = PIX // 512   # 8 column groups == psum banks == DMA tiles
ROWS_PER_BANK = 512 // W  # 8 image rows per psum bank
GPAD = 16         # gt inner stride (pad 2 channels to 16B for DoubleRow LDW)
AFF_BUFS = 4      # 2MB tiles in flight

_CACHE = {}


def _split_multi_waits(nc, limit=1):
    """The walrus build in this toolchain rejects instructions carrying more
    than one sync wait (any template: CTRL, S3_LW, ...). Tile's scheduler
    freely emits multi-wait instructions. Post-process the scheduled BIR:
    hoist excess waits onto one-wait NoOps inserted immediately before the
    instruction on the same engine (sequencer waits are conjunctive and
    blocking, so semantics are identical)."""
    import concourse.mybir as mybir

    n_split = 0
    for f in nc.m.functions:
        for b in f.blocks:
            insts = b.instructions  # live view
            i = 0
            while i < len(insts):
                inst = insts[i]
                si = inst.sync_info
                if si is not None and len(si.on_wait) > limit:
                    waits = list(si.on_wait)
                    extra, keep = waits[:-limit], waits[-limit:]
                    for w in extra:
                        nop = mybir.InstNoOp(name=f"SWS-{n_split}")
                        n_split += 1
                        nop.engine = inst.engine
                        nop.sync_info = mybir.SyncInfo(on_wait=[w], on_update=[])
                        insts.insert(i, nop)
                        i += 1
                    inst.sync_info = mybir.SyncInfo(
                        on_wait=keep, on_update=si.on_update
                    )
                i += 1
    return n_split


def _build_nc(split=True):
    import concourse.bass as bass
    import concourse.mybir as mybir
    import concourse.tile as tile

    f32 = mybir.dt.float32
    f8 = mybir.dt.float8e4
    nc = bass.Bass()
    # aff pre-packed on the host: [group, p, pair, i, n]; element
    # (row, col) of aff[b] with row = 256*u + 128*i + p, col = 512*g + n
    # lands at [g, p, u, i, n]. One contiguous 2MB transfer per group, except
    # the last group which streams as two 1MB halves so its matmul burst
    # (and the dependent final reductions) start ~2.5us earlier.
    aff = nc.dram_tensor("aff", [NT, 128, NPAIR, 2, 512], f8, kind="ExternalInput")
    # gt[p, kc, 0:2] = g[c, 128*kc + p]; inner dim padded to GPAD so the
    # DoubleRow weight pair stride is 16B
    gt = nc.dram_tensor("gt", [128, KC, GPAD], f8, kind="ExternalInput")
    out = nc.dram_tensor("out", [C, 4], f32, kind="ExternalOutput")

    with tile.TileContext(nc) as tc:
        with (
            tc.tile_pool(name="consts", bufs=1) as consts,
            tc.tile_pool(name="small", bufs=1) as small,
            tc.tile_pool(name="sqp", bufs=2) as sqp,
            tc.tile_pool(name="affp", bufs=AFF_BUFS) as affp,
            tc.tile_pool(name="ps1", bufs=1, space="PSUM") as ps1,
        ):
            # consts go through SWDGE (gpsimd) so they never queue behind the
            # big aff stream on the HWDGE rings
            gt_sb = consts.tile([128, KC, GPAD], f8)
            nc.gpsimd.dma_start(out=gt_sb, in_=gt[:, :, :])

            y_sb = small.tile([C, H, OW], f32)      # w-windowsums of x
            ysq_sb = small.tile([C, H, OW], f32)    # w-windowsums of x^2
            s_sb = small.tile([C, OH * OW], f32)    # full window sums
            ssq_sb = small.tile([C, OH * OW], f32)  # full window sums of x^2
            s2_sb = small.tile([C, OH * OW], f32)   # S^2
            # [ssq_lo, s2_lo, ssq_hi, s2_hi]: window rows 0-12 reduce early
            # (after bank 6), only rows 13-14 remain on the critical tail
            out_sb = small.tile([C, 4], f32)

            def windowed(ap, row_step, n_rows):
                """4-dim overlapping AP: [part, row, window j, dw] over a
                (C, n_rows*row_step) region; one tensor_reduce(X) gives the
                w-direction window sums in a single instruction."""
                return bass.AP(
                    tensor=ap.tensor,
                    offset=ap.offset,
                    ap=[list(ap.ap[0]), [row_step, n_rows], [STRIDE, OW], [1, WIN]],
                )

            def bank_postprocess(n, bank):
                """w-direction window sums for psum bank n; overlaps stream."""
                sq = sqp.tile([C, 512], f32, tag="sq")
                nc.scalar.square(out=sq, in_=bank)
                yd = y_sb[:, n * ROWS_PER_BANK:(n + 1) * ROWS_PER_BANK, :]
                qd = ysq_sb[:, n * ROWS_PER_BANK:(n + 1) * ROWS_PER_BANK, :]
                nc.vector.reduce_sum(
                    out=yd, in_=windowed(bank, W, ROWS_PER_BANK),
                    axis=mybir.AxisListType.X,
                )
                nc.vector.reduce_sum(
                    out=qd, in_=windowed(sq[:, :], W, ROWS_PER_BANK),
                    axis=mybir.AxisListType.X,
                )

            # h-direction window sums, incremental: S[c, i, j] = sum_dh
            # Y[c, 4i+dh, j]. Window row i needs Y rows 4i..4i+7; after bank
            # n the rows up to 8n+7 exist, so rows {2n-1, 2n} (and row 0 for
            # n=0) become computable.
            sv = s_sb.rearrange("c (i j) -> c i j", j=OW)
            qv = ssq_sb.rearrange("c (i j) -> c i j", j=OW)

            def h_rows(i0, cnt):
                for src, dst in ((y_sb, sv), (ysq_sb, qv)):
                    ap = src[:, :, :]
                    win = bass.AP(
                        tensor=ap.tensor,
                        offset=ap.offset + i0 * STRIDE * OW,
                        ap=[list(ap.ap[0]), [STRIDE * OW, cnt], [1, OW], [OW, WIN]],
                    )
                    nc.vector.reduce_sum(
                        out=dst[:, i0:i0 + cnt, :], in_=win,
                        axis=mybir.AxisListType.X,
                    )

            # one PSUM bank tile per column group; group g covers pixels
            # [512g, 512g+512) == image rows [8g, 8g+8)
            c1bs = [
                ps1.tile([C, 512], f32, tag="bank", bufs=NT, name=f"c1b{g}")
                for g in range(NT)
            ]
            HP = NPAIR // 2  # pairs per half-tile
            NLO = 13 * OW    # window rows 0-12, reduced early
            for g in range(NT):
                halves = [(0, NPAIR)] if g < NT - 1 else [(0, HP), (HP, NPAIR)]
                for u0, u1 in halves:
                    at = affp.tile([128, u1 - u0, 2, 512], f8, tag="aff")
                    nc.sync.dma_start(out=at, in_=aff[g, :, u0:u1])
                    for u in range(u0, u1):
                        nc.tensor.matmul(
                            c1bs[g],
                            lhsT=gt_sb[:, 2 * u:2 * u + 2, 0:2],
                            rhs=at[:, u - u0, :, :],
                            start=(u == 0),
                            stop=(u == NPAIR - 1),
                            perf_mode=mybir.MatmulPerfMode.DoubleRow,
                        )
                bank_postprocess(g, c1bs[g])
                if g == 0:
                    h_rows(0, 1)
                else:
                    h_rows(2 * g - 1, 2)
                if g == NT - 2:
                    # rows 0-12 of S/SSq are final: fold them down now so
                    # only rows 13-14 (30 windows) remain after bank 7
                    nc.scalar.square(out=s2_sb[:, :NLO], in_=s_sb[:, :NLO])
                    nc.vector.reduce_sum(
                        out=out_sb[:, 0:1], in_=ssq_sb[:, :NLO],
                        axis=mybir.AxisListType.X,
                    )
                    nc.vector.reduce_sum(
                        out=out_sb[:, 1:2], in_=s2_sb[:, :NLO],
                        axis=mybir.AxisListType.X,
                    )

            nc.scalar.square(out=s2_sb[:, NLO:], in_=s_sb[:, NLO:])
            nc.vector.reduce_sum(
                out=out_sb[:, 2:3], in_=ssq_sb[:, NLO:], axis=mybir.AxisListType.X
            )
            nc.vector.reduce_sum(
                out=out_sb[:, 3:4], in_=s2_sb[:, NLO:], axis=mybir.AxisListType.X
            )
            nc.sync.dma_start(out=out[:, :], in_=out_sb)
    if split:
        _split_multi_waits(nc)
    return nc


def _f8(x):
    import ml_dtypes

    return np.asarray(x, dtype=np.float32).astype(ml_dtypes.float8_e4m3)


def _gt_host(grid_b):
    # grid_b: (64, 64, 2). g[c, p] = grid_b.reshape(4096, 2)[p, c]
    # gt layout: gt[p, kc, c] = g[c, 128*kc + p], inner padded to GPAD
    import ml_dtypes

    g = np.ascontiguousarray(grid_b, dtype=np.float32).reshape(PIX, C)
    gt = np.zeros((128, KC, GPAD), dtype=ml_dtypes.float8_e4m3)
    gt[:, :, :C] = _f8(g.reshape(KC, 128, C).transpose(1, 0, 2))
    return gt


def _aff_host(aff_b):
    # pack into DMA-tile order [g, p, u, i, n]:
    # element (row, col) with row = 256u + 128i + p, col = 512g + n
    a8 = _f8(aff_b)  # (4096, 4096)
    a8 = a8.reshape(NPAIR, 2, 128, NT, 512).transpose(3, 2, 0, 1, 4)
    return np.ascontiguousarray(a8)


def run_cores(aff, grid, trace=False):
    """Compile (cached) and run the per-core bass kernel on cores 0..7.

    Returns the BassKernelResults from run_bass_kernel_spmd."""
    from concourse.bass_utils import run_bass_kernel_spmd

    if "nc" not in _CACHE:
        _CACHE["nc"] = _build_nc()
    nc = _CACHE["nc"]

    in_maps = []
    for b in range(B):
        in_maps.append({"aff": _aff_host(aff[b]), "gt": _gt_host(grid[b])})
    return run_bass_kernel_spmd(nc, in_maps, core_ids=list(range(B)), trace=trace)


def kernel(aff, grid):
    aff = np.asarray(aff, dtype=np.float32)
    grid = np.asarray(grid, dtype=np.float32)
    res = run_cores(aff, grid)
    total = 0.0
    for b in range(B):
        o = res.results[b]["out"].astype(np.float64)
        ssq = o[:, 0].sum() + o[:, 2].sum()
        s2 = o[:, 1].sum() + o[:, 3].sum()
        total += ssq - s2 / (WIN * WIN)
    total /= B * C * OH * OW * WIN * WIN
    return np.asarray(total, dtype=np.float32)
